# revision 1
# baseline (speedup 1.0000x reference)
"""Trainium2 Bass kernel for a single-head linear-projection attention block.

Reference computation (B=4, CH=256, N=4096):
    theta = Wt @ x        [B, 32, N]
    phi   = Wp @ x        [B, 32, N]
    g     = Wg @ x        [B, 128, N]
    scores = theta^T phi  [B, N, N]
    beta = softmax(scores, axis=-1)
    attn = g @ beta^T     [B, 128, N]
    out = gamma * (Wo @ attn) + x

Sharding: 8 cores = 4 batches x 2 query-halves. Each core owns one batch's
full sequence (for keys/values) and half the queries. The per-core x is
rotated so its query half is always columns 0:2048, keeping the SPMD program
identical across cores (softmax/attention are invariant to a consistent
permutation of the key axis). No collectives are needed.

Per-core dataflow (all matmuls bf16 with fp32 PSUM accumulation):
  - theta[32, 2048], phi[32, 4096] via weight-stationary matmuls.
  - gT[m, c] via x-stationary matmuls (g transposed, m on partitions), since
    the attention matmul contracts over m which must sit on partitions.
  - scoresT[m, n] = phi^T theta computed transposed so that softmax's exp
    output directly feeds the attention matmul without a transpose.
  - exp on the Scalar engine (no max subtraction needed: |scores| <~ 30).
  - attention accumulated over 32 m-tiles in PSUM; softmax denominator
    S[n] accumulated on the Vector engine and partition-reduced with a
    ones-matmul; normalization folded into the PSUM->SBUF drain.
  - gamma folded into Wo on the host; fp32 residual add with x.
"""

import os
import sys

import numpy as np

B, CH, N = 4, 256, 4096
NCORES = 8
NH = N // 2  # queries per core
P = 128

_REPO_CANDIDATES = ["/opt/trn_rl_repo", "/root/.axon_site/_ro/trn_rl_repo"]


def _ensure_import_path():
    try:
        import concourse.bass  # noqa: F401
        return
    except ImportError:
        pass
    for cand in _REPO_CANDIDATES:
        if os.path.isdir(cand):
            sys.path.insert(0, cand)
            try:
                import concourse.bass  # noqa: F401
                return
            except ImportError:
                sys.path.pop(0)
    raise ImportError("could not locate concourse (bass) repo")


_CACHE = {}


def build_bass():
    """Build + compile the per-core Tile program (identical on all 8 cores)."""
    _ensure_import_path()
    import concourse.bacc as bacc
    import concourse.tile as tile
    from concourse import mybir

    dt = mybir.dt
    f32 = dt.float32
    bf16 = dt.bfloat16
    Exp = mybir.ActivationFunctionType.Exp

    nc = bacc.Bacc(
        "TRN2",
        target_bir_lowering=False,
        debug=False,
        num_devices=NCORES,
    )

    # Per-core DRAM I/O.
    x_d = nc.dram_tensor("x", [CH, N], bf16, kind="ExternalInput")
    xq_d = nc.dram_tensor("xq", [CH, NH], f32, kind="ExternalInput")
    wt_d = nc.dram_tensor("wt", [CH, 32], bf16, kind="ExternalInput")   # Wt^T
    wp_d = nc.dram_tensor("wp", [CH, 32], bf16, kind="ExternalInput")   # Wp^T
    wg_d = nc.dram_tensor("wg", [CH, 128], bf16, kind="ExternalInput")  # Wg^T
    wo_d = nc.dram_tensor("wo", [128, CH], bf16, kind="ExternalInput")  # (gamma*Wo)^T
    out_d = nc.dram_tensor("out", [CH, NH], f32, kind="ExternalOutput")

    MT = N // P  # 32 m-tiles

    with tile.TileContext(nc) as tc:
        with (
            tc.tile_pool(name="const", bufs=1) as const,
            tc.tile_pool(name="xp", bufs=1) as xp,
            tc.tile_pool(name="proj", bufs=1) as proj,
            tc.tile_pool(name="expp", bufs=4) as expp,
            tc.tile_pool(name="acc", bufs=1) as acc,
            tc.tile_pool(name="ps2", bufs=2, space="PSUM") as ps2,
            tc.tile_pool(name="psA", bufs=1, space="PSUM") as psA,
        ):
            # ---- constants / weights ----
            wt_sb = const.tile([P, 2, 32], bf16)
            wp_sb = const.tile([P, 2, 32], bf16)
            wg_sb = const.tile([P, 2, 128], bf16)
            wo_sb = const.tile([P, CH], bf16)
            ones_sb = const.tile([P, P], bf16)
            nc.sync.dma_start(
                out=wt_sb, in_=wt_d.ap().rearrange("(kb p) m -> p kb m", p=P)
            )
            nc.sync.dma_start(
                out=wp_sb, in_=wp_d.ap().rearrange("(kb p) m -> p kb m", p=P)
            )
            nc.sync.dma_start(
                out=wg_sb, in_=wg_d.ap().rearrange("(kb p) m -> p kb m", p=P)
            )
            nc.sync.dma_start(out=wo_sb, in_=wo_d.ap())
            nc.vector.memset(ones_sb, 1.0)

            # ---- x into SBUF (bf16 for matmuls, fp32 query-slice for residual) ----
            x_sb = xp.tile([P, 2, N], bf16)
            xq_sb = xp.tile([P, 2, NH], f32)
            for kb in range(2):
                nc.sync.dma_start(
                    out=x_sb[:, kb, :], in_=x_d[kb * P:(kb + 1) * P, :]
                )
                nc.sync.dma_start(
                    out=xq_sb[:, kb, :], in_=xq_d[kb * P:(kb + 1) * P, :]
                )

            # ---- projections: theta [32, NH], phi [32, N] (both partitions 0:32) ----
            theta_sb = proj.tile([32, NH], bf16)
            phi_sb = proj.tile([32, N], bf16)
            for half in range(2):
                ps_t = ps2.tile([32, 1024], f32, tag="ps")
                for c in range(2):
                    sl = slice(half * 1024 + c * 512, half * 1024 + (c + 1) * 512)
                    for kb in range(2):
                        nc.tensor.matmul(
                            ps_t[:, c * 512:(c + 1) * 512],
                            lhsT=wt_sb[:, kb, :],
                            rhs=x_sb[:, kb, sl],
                            start=(kb == 0),
                            stop=(kb == 1),
                        )
                nc.vector.tensor_copy(
                    out=theta_sb[:, half * 1024:(half + 1) * 1024], in_=ps_t
                )
            for quarter in range(4):
                ps_p = ps2.tile([32, 1024], f32, tag="ps")
                for c in range(2):
                    sl = slice(quarter * 1024 + c * 512, quarter * 1024 + (c + 1) * 512)
                    for kb in range(2):
                        nc.tensor.matmul(
                            ps_p[:, c * 512:(c + 1) * 512],
                            lhsT=wp_sb[:, kb, :],
                            rhs=x_sb[:, kb, sl],
                            start=(kb == 0),
                            stop=(kb == 1),
                        )
                nc.vector.tensor_copy(
                    out=phi_sb[:, quarter * 1024:(quarter + 1) * 1024], in_=ps_p
                )

            # ---- gT [m, c]: x-stationary matmuls, 4 m-tiles per PSUM drain ----
            gT_sb = proj.tile([P, MT, P], bf16)
            for grp in range(MT // 4):
                ps_g = ps2.tile([P, 4, P], f32, tag="ps")
                for j in range(4):
                    mt = grp * 4 + j
                    for kb in range(2):
                        nc.tensor.matmul(
                            ps_g[:, j, :],
                            lhsT=x_sb[:, kb, mt * P:(mt + 1) * P],
                            rhs=wg_sb[:, kb, :],
                            start=(kb == 0),
                            stop=(kb == 1),
                        )
                nc.vector.tensor_copy(out=gT_sb[:, grp * 4:(grp + 1) * 4, :], in_=ps_g)

            # ---- main loop: scoresT -> exp -> {attention accumulate, S accumulate} ----
            S_part = acc.tile([P, NH], f32)
            attn_ps = psA.tile([P, NH], f32)
            for mt in range(MT):
                for nh in range(2):
                    ps_s = ps2.tile([P, 1024], f32, tag="ps")
                    for c in range(2):
                        sl = slice(nh * 1024 + c * 512, nh * 1024 + (c + 1) * 512)
                        nc.tensor.matmul(
                            ps_s[:, c * 512:(c + 1) * 512],
                            lhsT=phi_sb[:, mt * P:(mt + 1) * P],
                            rhs=theta_sb[:, sl],
                            start=True,
                            stop=True,
                        )
                    expt = expp.tile([P, 1024], bf16, tag="expt")
                    nc.scalar.activation(out=expt, in_=ps_s, func=Exp)
                    for c in range(2):
                        sl = slice(nh * 1024 + c * 512, nh * 1024 + (c + 1) * 512)
                        nc.tensor.matmul(
                            attn_ps[:, sl],
                            lhsT=gT_sb[:, mt, :],
                            rhs=expt[:, c * 512:(c + 1) * 512],
                            start=(mt == 0),
                            stop=(mt == MT - 1),
                        )
                    dst = S_part[:, nh * 1024:(nh + 1) * 1024]
                    if mt == 0:
                        nc.vector.tensor_copy(out=dst, in_=expt)
                    else:
                        nc.vector.tensor_add(dst, dst, expt)

            # ---- softmax denominator: partition-reduce S via ones-matmul ----
            S_bf = acc.tile([P, NH], bf16)
            nc.vector.tensor_copy(out=S_bf, in_=S_part)
            recip = acc.tile([P, NH], f32)
            for nh in range(2):
                ps_b = ps2.tile([P, 1024], f32, tag="ps")
                for c in range(2):
                    sl = slice(nh * 1024 + c * 512, nh * 1024 + (c + 1) * 512)
                    nc.tensor.matmul(
                        ps_b[:, c * 512:(c + 1) * 512],
                        lhsT=ones_sb,
                        rhs=S_bf[:, sl],
                        start=True,
                        stop=True,
                    )
                nc.vector.reciprocal(
                    out=recip[:, nh * 1024:(nh + 1) * 1024], in_=ps_b
                )

            # ---- normalize attention, project with (gamma*Wo), add residual ----
            A_norm = acc.tile([P, NH], bf16)
            for nh in range(2):
                sl = slice(nh * 1024, (nh + 1) * 1024)
                nc.vector.tensor_mul(A_norm[:, sl], attn_ps[:, sl], recip[:, sl])

            out_sb = acc.tile([P, 2, NH], f32)
            for oc in range(2):
                for nh in range(2):
                    ps_o = ps2.tile([P, 1024], f32, tag="ps")
                    for c in range(2):
                        sl = slice(nh * 1024 + c * 512, nh * 1024 + (c + 1) * 512)
                        nc.tensor.matmul(
                            ps_o[:, c * 512:(c + 1) * 512],
                            lhsT=wo_sb[:, oc * P:(oc + 1) * P],
                            rhs=A_norm[:, sl],
                            start=True,
                            stop=True,
                        )
                    sl = slice(nh * 1024, (nh + 1) * 1024)
                    nc.vector.tensor_add(
                        out_sb[:, oc, sl], ps_o, xq_sb[:, oc, sl]
                    )
                nc.sync.dma_start(
                    out=out_d[oc * P:(oc + 1) * P, :], in_=out_sb[:, oc, :]
                )

    nc.compile()
    return nc


def get_nc():
    if "nc" not in _CACHE:
        _CACHE["nc"] = build_bass()
    return _CACHE["nc"]


def make_in_maps(x, Wt, Wp, Wg, Wo, gamma):
    import ml_dtypes

    bf16 = ml_dtypes.bfloat16
    x = np.asarray(x, dtype=np.float32)
    wt = np.ascontiguousarray(np.asarray(Wt, np.float32).T).astype(bf16)
    wp = np.ascontiguousarray(np.asarray(Wp, np.float32).T).astype(bf16)
    wg = np.ascontiguousarray(np.asarray(Wg, np.float32).T).astype(bf16)
    wo = np.ascontiguousarray(
        (float(np.asarray(gamma)) * np.asarray(Wo, np.float32)).T
    ).astype(bf16)
    in_maps = []
    for i in range(NCORES):
        b, h = divmod(i, 2)
        xb = x[b]
        if h:
            xb = np.concatenate([xb[:, NH:], xb[:, :NH]], axis=1)
        in_maps.append(
            {
                "x": np.ascontiguousarray(xb).astype(bf16),
                "xq": np.ascontiguousarray(x[b][:, h * NH:(h + 1) * NH]),
                "wt": wt,
                "wp": wp,
                "wg": wg,
                "wo": wo,
            }
        )
    return in_maps


def gather_out(results):
    out = np.empty((B, CH, N), np.float32)
    for i in range(NCORES):
        b, h = divmod(i, 2)
        out[b][:, h * NH:(h + 1) * NH] = results[i]["out"]
    return out


def kernel(x, Wt, Wp, Wg, Wo, gamma):
    _ensure_import_path()
    from concourse.bass_utils import run_bass_kernel_spmd

    nc = get_nc()
    in_maps = make_in_maps(x, Wt, Wp, Wg, Wo, gamma)
    res = run_bass_kernel_spmd(nc, in_maps, core_ids=list(range(NCORES)))
    return gather_out(res.results)
